# revision 3
# baseline (speedup 1.0000x reference)
"""Trainium2 Bass kernel for nn_AttentionLayer_10995116278518.

Computes softmax(einsum('sbe,e->bs', embedded, attn[:300])
              + einsum('sbf,f->bs', lstm_outputs, attn[300:]), axis=1)
(the reference's mask is computed-but-discarded, so it is unused here).

Sharding: data-parallel over batch. Each of the 8 cores handles 8 of the
64 batch rows; no cross-device communication.

The kernel is HBM/DMA-engine-bandwidth bound (~36 MB/core at fp16, the
16 SDMA engines sustain ~26 GB/s each), so everything is built around
clean DMA streaming with minimal head/tail overhead:
  - host concatenates embedded+lstm features, casts to fp16 (validated
    against the 2e-2 tolerance; bf16 is NOT accurate enough), and lays
    the shard out feature-major, pre-permuted so every DMA is a single
    fully-contiguous read with large per-partition descriptor runs.
  - the stream is 18 DMAs (4 MiB groups early for descriptor
    efficiency, shrinking to 0.25 MiB column-quarters at the end so the
    PE never has a large backlog when the last byte lands), split
    across the two HWDGE rings with cumulative bytes balanced to 16 KB
    so both rings finish together.
  - TensorE does the dots with a windowed-attn lhsT: tile T[k, c, j]
    holds attn chunk c at column 8, zeros elsewhere; the lhsT for
    (chunk c, batch row b) is the 8-column window T[:, c, 8-b : 16-b],
    which places the attn column at output row b. ALL matmuls therefore
    accumulate into a single PSUM bank whose partition b holds batch
    row b's logits -- no PSUM->SBUF copies, no scatter DMAs.
  - matmuls are ordered by estimated DMA completion; the softmax
    (max/exp/sum/scale) reads the PSUM bank directly.
"""

import sys

import numpy as np

try:
    import concourse.bass as bass
except ImportError:  # stand-alone grading dir: the runtime lives here
    sys.path.insert(0, "/opt/trn_rl_repo")
    import concourse.bass as bass

import concourse.bacc as bacc
import concourse.tile as tile
from concourse import mybir
from concourse.bass_utils import run_bass_kernel_spmd

SEQ = 512
BATCH = 64
EMB = 300
LSTM = 4096
D = EMB + LSTM  # 4396
N_CORES = 8
BLOC = BATCH // N_CORES  # 8 batch rows per core
P = 128
RC = BLOC * SEQ  # 4096 columns (b-major) per chunk
NCH = (D + P - 1) // P  # 35 feature chunks: 34 full + 1 of 44
KLAST = D - (NCH - 1) * P  # 44

F32 = mybir.dt.float32
F16 = mybir.dt.float16

# windowed-attn tile: T[k, c, j], attn chunk c at column WCOL, so the
# [*, 8] window starting at WCOL-b has the attn column at index b
WSTRIDE = 16
WCOL = 8

CPART = 34  # the 44-row partial chunk
CLAST = 33  # streamed last, in four column-quarters (2 batch rows each)

# stream schedule: (queue, kind, payload). Queue bytes are balanced to
# 16 KB (sync 17.25 MiB vs scalar 17.234 MiB incl. the attn tile).
SCHED = [
    ("sy", "g", [0, 1, 2, 3]),
    ("sc", "g", [4, 5, 6, 7]),
    ("sy", "g", [8, 9, 10, 11]),
    ("sc", "g", [12, 13, 14, 15]),
    ("sy", "g", [16, 17, 18, 19]),
    ("sc", "g", [20, 21, 22, 23]),
    ("sy", "g", [24, 25]),
    ("sy", "g", [26, 27]),
    ("sc", "g", [29]),
    ("sc", "g", [30]),
    ("sc", "g", [31]),
    ("sc", "g", [32]),
    ("sc", "p", CPART),
    ("sy", "g", [28]),
    ("sc", "q", 1),
    ("sc", "q", 2),
    ("sy", "q", 0),
    ("sc", "q", 3),
]

# matmul processing order, matching estimated DMA completion order
PE_ORDER = (
    [("c", c) for c in range(24)]
    + [("c", 29), ("c", 24), ("c", 25), ("c", 30), ("c", 31), ("c", 26),
       ("c", 27), ("c", 32), ("c", CPART), ("c", 28),
       ("q", 1), ("q", 2), ("q", 0), ("q", 3)]
)

# chunks that get a trailing zero-weight keep-warm matmul
DUMMY_CHUNKS = set(range(28))


def _build() -> bass.Bass:
    nc = bacc.Bacc()
    # flat fp16 stream, pre-permuted on host to match the DMA schedule
    x = nc.declare_dram_parameter("x", [D * RC], F16, isOutput=False)
    attn_win = nc.declare_dram_parameter(
        "attn_win", [P, (NCH + 1) * WSTRIDE], F16, isOutput=False
    )
    out = nc.declare_dram_parameter("out", [BLOC, SEQ], F16, isOutput=True)

    # flat element offsets of each transfer in x, in SCHED order
    offs = {}
    pos = 0
    for _, kind, pl in SCHED:
        key = (kind, tuple(pl) if isinstance(pl, list) else pl)
        offs[key] = pos
        if kind == "g":
            pos += len(pl) * P * RC
        elif kind == "p":
            pos += KLAST * RC
        else:
            pos += P * (RC // 4)
    assert pos == D * RC

    with tile.TileContext(nc) as tc:
        with (
            tc.tile_pool(name="singles", bufs=1) as singles,
            tc.tile_pool(name="gpool4", bufs=3) as gpool4,
            tc.tile_pool(name="gpool2", bufs=2) as gpool2,
            tc.tile_pool(name="tpool", bufs=7) as tpool,
            tc.tile_pool(name="psum", bufs=1, space="PSUM") as psum_pool,
        ):
            sb_attn = singles.tile([P, NCH + 1, WSTRIDE], F16)
            nc.scalar.dma_start(out=sb_attn, in_=attn_win[:, :])

            ps = psum_pool.tile([BLOC, SEQ], F32)

            # issue the whole stream up front; the tile pools throttle
            # via buffer-recycle semaphores
            where = {}  # chunk -> (tile, col0)
            lt = None
            for qn, kind, pl in SCHED:
                eng = nc.sync if qn == "sy" else nc.scalar
                key = (kind, tuple(pl) if isinstance(pl, list) else pl)
                a = offs[key]
                if kind == "g":
                    n = len(pl)
                    pool = {4: gpool4, 2: gpool2, 1: tpool}[n]
                    gt = pool.tile([P, n * RC], F16, tag=f"b{n}")
                    eng.dma_start(out=gt, in_=x[a : a + n * P * RC])
                    for j, c in enumerate(pl):
                        where[c] = (gt, j * RC)
                elif kind == "p":
                    pt = tpool.tile([P, RC], F16, tag="b1")
                    eng.dma_start(out=pt[0:KLAST, :], in_=x[a : a + KLAST * RC])
                    where[CPART] = (pt, 0)
                else:
                    q = pl
                    if lt is None:
                        lt = tpool.tile([P, RC], F16, tag="b1")
                        where[CLAST] = (lt, 0)
                    eng.dma_start(
                        out=lt[:, q * (RC // 4) : (q + 1) * (RC // 4)],
                        in_=x[a : a + P * (RC // 4)],
                    )

            first = True

            def mm(c, b, kp, xt, col0, stop=False):
                nonlocal first
                nc.tensor.matmul(
                    out=ps,
                    lhsT=sb_attn[0:kp, c, WCOL - b : WCOL - b + BLOC],
                    rhs=xt[0:kp, col0 + b * SEQ : col0 + (b + 1) * SEQ],
                    start=first,
                    stop=stop,
                    skip_group_check=True,
                )
                first = False

            n_items = len(PE_ORDER)
            for i, (kind, v) in enumerate(PE_ORDER):
                last_item = i == n_items - 1
                if kind == "c":
                    kp = KLAST if v == CPART else P
                    xt, col0 = where[v]
                    for b in range(BLOC):
                        mm(v, b, kp, xt, col0,
                           stop=last_item and b == BLOC - 1)
                    if v in DUMMY_CHUNKS:
                        # zero-weight matmul: adds 0 but keeps the PE
                        # HAM clock gate warm through DMA-wait gaps
                        nc.tensor.matmul(
                            out=ps,
                            lhsT=sb_attn[0:P, NCH, 0:BLOC],
                            rhs=xt[0:P, col0 : col0 + SEQ],
                            start=False,
                            stop=False,
                            skip_group_check=True,
                        )
                else:  # column-quarter q of CLAST: batch rows 2q, 2q+1
                    xt, col0 = where[CLAST]
                    for b in (2 * v, 2 * v + 1):
                        mm(CLAST, b, P, xt, col0,
                           stop=last_item and b == 2 * v + 1)

            # softmax along s (free axis), reading logits straight from
            # PSUM: partition b of the bank holds batch row b
            nm = singles.tile([BLOC, 1], F32)
            ssum = singles.tile([BLOC, 1], F32)
            rec = singles.tile([BLOC, 1], F32)
            expt = singles.tile([BLOC, SEQ], F32)
            res = singles.tile([BLOC, SEQ], F16)
            nc.vector.tensor_reduce(
                out=nm,
                in_=ps,
                axis=mybir.AxisListType.X,
                op=mybir.AluOpType.max,
                negate=True,
            )
            nc.scalar.activation(
                out=expt,
                in_=ps,
                func=mybir.ActivationFunctionType.Exp,
                bias=nm,
                scale=1.0,
                accum_out=ssum,
            )
            nc.vector.reciprocal(rec, ssum)
            nc.vector.tensor_scalar_mul(res, expt, rec)
            nc.sync.dma_start(out=out[:, :], in_=res)

    nc.compile()
    return nc


_NC_CACHE = None


def _get_nc() -> bass.Bass:
    global _NC_CACHE
    if _NC_CACHE is None:
        _NC_CACHE = _build()
    return _NC_CACHE


def _make_in_maps(embedded, lstm_outputs, attn):
    embedded = np.asarray(embedded, dtype=np.float32)
    lstm_outputs = np.asarray(lstm_outputs, dtype=np.float32)
    attn = np.asarray(attn, dtype=np.float32).astype(np.float16)
    # [S, B, F] -> [s, core, b, F]
    emb4 = embedded.reshape(SEQ, N_CORES, BLOC, EMB)
    lst4 = lstm_outputs.reshape(SEQ, N_CORES, BLOC, LSTM)
    att_win = np.zeros((P, NCH + 1, WSTRIDE), dtype=np.float16)
    for c in range(NCH):
        kp = P if c < NCH - 1 else KLAST
        att_win[:kp, c, WCOL] = attn[c * P : c * P + kp]
    att_flat = att_win.reshape(P, (NCH + 1) * WSTRIDE)
    in_maps = []
    for i in range(N_CORES):
        xs = np.empty((D, RC), dtype=np.float16)
        # [s, b, F] -> [F, b, s] -> [F, b*512+s]
        xs[:EMB] = emb4[:, i].transpose(2, 1, 0).reshape(EMB, RC)
        xs[EMB:] = lst4[:, i].transpose(2, 1, 0).reshape(LSTM, RC)
        x33 = xs[CLAST * P : (CLAST + 1) * P].reshape(P, 4, RC // 4)
        pieces = []
        for _, kind, pl in SCHED:
            if kind == "g":
                c0, n = pl[0], len(pl)
                pieces.append(
                    xs[c0 * P : (c0 + n) * P]
                    .reshape(n, P, RC)
                    .transpose(1, 0, 2)
                    .ravel()
                )
            elif kind == "p":
                pieces.append(xs[CPART * P : CPART * P + KLAST].ravel())
            else:
                pieces.append(x33[:, pl, :].ravel())
        in_maps.append({"x": np.concatenate(pieces), "attn_win": att_flat})
    return in_maps


def _run(embedded, lstm_outputs, attn, trace=False, **spmd_kwargs):
    nc = _get_nc()
    in_maps = _make_in_maps(embedded, lstm_outputs, attn)
    r = run_bass_kernel_spmd(
        nc, in_maps, core_ids=list(range(N_CORES)), trace=trace, **spmd_kwargs
    )
    out = np.concatenate([r.results[i]["out"] for i in range(N_CORES)], axis=0)
    return out, r


def kernel(embedded, lstm_outputs, attn, mask=None, **_ignored) -> np.ndarray:
    out, _ = _run(embedded, lstm_outputs, attn, trace=False)
    return out.astype(np.float32)


# revision 7
# speedup vs baseline: 1.3572x; 1.3572x over previous
"""Trainium2 Bass kernel for nn_AttentionLayer_10995116278518.

Computes softmax(einsum('sbe,e->bs', embedded, attn[:300])
              + einsum('sbf,f->bs', lstm_outputs, attn[300:]), axis=1)
(the reference's mask is computed-but-discarded, so it is unused here).

Sharding: data-parallel over batch. Each of the 8 cores handles 8 of the
64 batch rows; no cross-device communication.

The kernel is HBM/DMA-engine-bandwidth bound (~36 MB/core at fp16, the
16 SDMA engines sustain ~26 GB/s each), so everything is built around
clean DMA streaming with minimal head/tail overhead:
  - host concatenates embedded+lstm features, casts to fp16 (validated
    against the 2e-2 tolerance; bf16 is NOT accurate enough), and lays
    the shard out feature-major, pre-permuted so every DMA is a single
    fully-contiguous read with large per-partition descriptor runs.
  - the stream is 17 DMAs (4 MiB groups early for descriptor
    efficiency, shrinking to 0.25 MiB column-quarters at the end so the
    PE never has a large backlog when the last byte lands), all on ONE
    HWDGE ring: a single ring saturates all 16 SDMA engines, and
    completion order is then exactly FIFO so the matmul schedule never
    guesses cross-ring interleaving.
  - TensorE does the dots with a windowed-attn lhsT: tile T[k, c, j]
    holds attn chunk c at column 8, zeros elsewhere; the lhsT for
    (chunk c, batch row b) is the 8-column window T[:, c, 8-b : 16-b],
    which places the attn column at output row b. ALL matmuls therefore
    accumulate into a single PSUM bank whose partition b holds batch
    row b's logits -- no PSUM->SBUF copies, no scatter DMAs.
  - matmuls are ordered by estimated DMA completion; the softmax
    (max/exp/sum/scale) reads the PSUM bank directly.
"""

import sys

import numpy as np

try:
    import concourse.bass as bass
except ImportError:  # stand-alone grading dir: the runtime lives here
    sys.path.insert(0, "/opt/trn_rl_repo")
    import concourse.bass as bass

import concourse.bacc as bacc
import concourse.tile as tile
from concourse import mybir
from concourse.bass_utils import run_bass_kernel_spmd

SEQ = 512
BATCH = 64
EMB = 300
LSTM = 4096
D = EMB + LSTM  # 4396
N_CORES = 8
BLOC = BATCH // N_CORES  # 8 batch rows per core
P = 128
RC = BLOC * SEQ  # 4096 columns (b-major) per chunk
NCH = (D + P - 1) // P  # 35 feature chunks: 34 full + 1 of 44
KLAST = D - (NCH - 1) * P  # 44

F32 = mybir.dt.float32
F16 = mybir.dt.float16

# windowed-attn tile: T[k, c, j], attn chunk c at column WCOL, so the
# [*, 8] window starting at WCOL-b has the attn column at index b
WSTRIDE = 16
WCOL = 8

CPART = 34  # the 44-row partial chunk
CLAST = 33  # streamed last, in four column-quarters (2 batch rows each)

# stream schedule: (queue, kind, payload). The whole x stream rides ONE
# HWDGE ring (sync): a single ring saturates all 16 SDMA engines, and
# completion order is then exactly FIFO, so the matmul order below never
# guesses cross-ring interleaving (the engines do NOT timeshare rings
# fairly -- a big backlog on one ring starves the other for tens of us).
# The attn tile and the output ride the otherwise-idle scalar ring.
SCHED = [
    ("sy", "g", [0, 1, 2, 3]),
    ("sy", "g", [4, 5, 6, 7]),
    ("sy", "g", [8, 9, 10, 11]),
    ("sy", "g", [12, 13, 14, 15]),
    ("sy", "g", [16, 17, 18, 19]),
    ("sy", "g", [20, 21, 22, 23]),
    ("sy", "g", [24, 25, 26, 27]),
    ("sy", "g", [28]),
    ("sy", "g", [29]),
    ("sy", "g", [30]),
    ("sy", "g", [31]),
    ("sy", "g", [32]),
    ("sy", "p", CPART),
    ("sy", "q", 0),
    ("sy", "q", 1),
    ("sy", "q", 2),
    ("sy", "q", 3),
]

# matmul processing order = the stream's FIFO completion order
PE_ORDER = (
    [("c", c) for c in range(33)]
    + [("c", CPART), ("q", 0), ("q", 1), ("q", 2), ("q", 3)]
)

# chunks that get a trailing zero-weight keep-warm matmul
DUMMY_CHUNKS = set(range(28))


def _build() -> bass.Bass:
    nc = bacc.Bacc()
    # flat fp16 stream, pre-permuted on host to match the DMA schedule
    x = nc.declare_dram_parameter("x", [D * RC], F16, isOutput=False)
    attn_win = nc.declare_dram_parameter(
        "attn_win", [P, (NCH + 1) * WSTRIDE], F16, isOutput=False
    )
    out = nc.declare_dram_parameter("out", [BLOC, SEQ], F16, isOutput=True)

    # flat element offsets of each transfer in x, in SCHED order
    offs = {}
    pos = 0
    for _, kind, pl in SCHED:
        key = (kind, tuple(pl) if isinstance(pl, list) else pl)
        offs[key] = pos
        if kind == "g":
            pos += len(pl) * P * RC
        elif kind == "p":
            pos += KLAST * RC
        else:
            pos += P * (RC // 4)
    assert pos == D * RC

    with tile.TileContext(nc) as tc:
        with (
            tc.tile_pool(name="singles", bufs=1) as singles,
            tc.tile_pool(name="gpool4", bufs=4) as gpool4,
            tc.tile_pool(name="tpool", bufs=7) as tpool,
            tc.tile_pool(name="psum", bufs=1, space="PSUM") as psum_pool,
        ):
            sb_attn = singles.tile([P, NCH + 1, WSTRIDE], F16)
            nc.scalar.dma_start(out=sb_attn, in_=attn_win[:, :])

            ps = psum_pool.tile([BLOC, SEQ], F32)

            # issue the whole stream up front; the tile pools throttle
            # via buffer-recycle semaphores
            where = {}  # chunk -> (tile, col0)
            lt = None
            for qn, kind, pl in SCHED:
                eng = nc.sync if qn == "sy" else nc.scalar
                key = (kind, tuple(pl) if isinstance(pl, list) else pl)
                a = offs[key]
                if kind == "g":
                    n = len(pl)
                    pool = gpool4 if n == 4 else tpool
                    gt = pool.tile([P, n * RC], F16, tag=f"b{n}")
                    eng.dma_start(out=gt, in_=x[a : a + n * P * RC])
                    for j, c in enumerate(pl):
                        where[c] = (gt, j * RC)
                elif kind == "p":
                    pt = tpool.tile([P, RC], F16, tag="b1")
                    eng.dma_start(out=pt[0:KLAST, :], in_=x[a : a + KLAST * RC])
                    where[CPART] = (pt, 0)
                else:
                    q = pl
                    if lt is None:
                        lt = tpool.tile([P, RC], F16, tag="b1")
                        where[CLAST] = (lt, 0)
                    eng.dma_start(
                        out=lt[:, q * (RC // 4) : (q + 1) * (RC // 4)],
                        in_=x[a : a + P * (RC // 4)],
                    )

            first = True

            def mm(c, b, kp, xt, col0, stop=False):
                nonlocal first
                nc.tensor.matmul(
                    out=ps,
                    lhsT=sb_attn[0:kp, c, WCOL - b : WCOL - b + BLOC],
                    rhs=xt[0:kp, col0 + b * SEQ : col0 + (b + 1) * SEQ],
                    start=first,
                    stop=stop,
                    skip_group_check=True,
                )
                first = False

            n_items = len(PE_ORDER)
            for i, (kind, v) in enumerate(PE_ORDER):
                last_item = i == n_items - 1
                if kind == "c":
                    kp = KLAST if v == CPART else P
                    xt, col0 = where[v]
                    for b in range(BLOC):
                        mm(v, b, kp, xt, col0,
                           stop=last_item and b == BLOC - 1)
                    if v in DUMMY_CHUNKS:
                        # zero-weight matmul: adds 0 but keeps the PE
                        # HAM clock gate warm through DMA-wait gaps
                        nc.tensor.matmul(
                            out=ps,
                            lhsT=sb_attn[0:P, NCH, 0:BLOC],
                            rhs=xt[0:P, col0 : col0 + SEQ],
                            start=False,
                            stop=False,
                            skip_group_check=True,
                        )
                else:  # column-quarter q of CLAST: batch rows 2q, 2q+1
                    xt, col0 = where[CLAST]
                    for b in (2 * v, 2 * v + 1):
                        mm(CLAST, b, P, xt, col0,
                           stop=last_item and b == 2 * v + 1)

            # softmax along s (free axis), reading logits straight from
            # PSUM: partition b of the bank holds batch row b
            nm = singles.tile([BLOC, 1], F32)
            ssum = singles.tile([BLOC, 1], F32)
            rec = singles.tile([BLOC, 1], F32)
            expt = singles.tile([BLOC, SEQ], F32)
            res = singles.tile([BLOC, SEQ], F16)
            nc.vector.tensor_reduce(
                out=nm,
                in_=ps,
                axis=mybir.AxisListType.X,
                op=mybir.AluOpType.max,
                negate=True,
            )
            nc.scalar.activation(
                out=expt,
                in_=ps,
                func=mybir.ActivationFunctionType.Exp,
                bias=nm,
                scale=1.0,
                accum_out=ssum,
            )
            nc.vector.reciprocal(rec, ssum)
            nc.vector.tensor_scalar_mul(res, expt, rec)
            nc.sync.dma_start(out=out[:, :], in_=res)

    nc.compile()
    return nc


_NC_CACHE = None


def _get_nc() -> bass.Bass:
    global _NC_CACHE
    if _NC_CACHE is None:
        _NC_CACHE = _build()
    return _NC_CACHE


def _make_in_maps(embedded, lstm_outputs, attn):
    embedded = np.asarray(embedded, dtype=np.float32)
    lstm_outputs = np.asarray(lstm_outputs, dtype=np.float32)
    attn = np.asarray(attn, dtype=np.float32).astype(np.float16)
    # [S, B, F] -> [s, core, b, F]
    emb4 = embedded.reshape(SEQ, N_CORES, BLOC, EMB)
    lst4 = lstm_outputs.reshape(SEQ, N_CORES, BLOC, LSTM)
    att_win = np.zeros((P, NCH + 1, WSTRIDE), dtype=np.float16)
    for c in range(NCH):
        kp = P if c < NCH - 1 else KLAST
        att_win[:kp, c, WCOL] = attn[c * P : c * P + kp]
    att_flat = att_win.reshape(P, (NCH + 1) * WSTRIDE)
    in_maps = []
    for i in range(N_CORES):
        xs = np.empty((D, RC), dtype=np.float16)
        # [s, b, F] -> [F, b, s] -> [F, b*512+s]
        xs[:EMB] = emb4[:, i].transpose(2, 1, 0).reshape(EMB, RC)
        xs[EMB:] = lst4[:, i].transpose(2, 1, 0).reshape(LSTM, RC)
        x33 = xs[CLAST * P : (CLAST + 1) * P].reshape(P, 4, RC // 4)
        pieces = []
        for _, kind, pl in SCHED:
            if kind == "g":
                c0, n = pl[0], len(pl)
                pieces.append(
                    xs[c0 * P : (c0 + n) * P]
                    .reshape(n, P, RC)
                    .transpose(1, 0, 2)
                    .ravel()
                )
            elif kind == "p":
                pieces.append(xs[CPART * P : CPART * P + KLAST].ravel())
            else:
                pieces.append(x33[:, pl, :].ravel())
        in_maps.append({"x": np.concatenate(pieces), "attn_win": att_flat})
    return in_maps


def _run(embedded, lstm_outputs, attn, trace=False, **spmd_kwargs):
    nc = _get_nc()
    in_maps = _make_in_maps(embedded, lstm_outputs, attn)
    r = run_bass_kernel_spmd(
        nc, in_maps, core_ids=list(range(N_CORES)), trace=trace, **spmd_kwargs
    )
    out = np.concatenate([r.results[i]["out"] for i in range(N_CORES)], axis=0)
    return out, r


def kernel(embedded, lstm_outputs, attn, mask=None, **_ignored) -> np.ndarray:
    out, _ = _run(embedded, lstm_outputs, attn, trace=False)
    return out.astype(np.float32)
